# revision 18
# baseline (speedup 1.0000x reference)
"""Trainium2 Bass kernel for STSBaselineNet (embed -> biLSTM -> max-pool).

Sharding v4: one LSTM direction per core (cores 0-3 fwd, 4-7 bwd, 64
sentences each, split into two independent 32-sentence chains that
ping-pong through the engines). The input projection emb @ W_ih^T + b is
folded on the host into a [V+1, 1024] bf16 gather table per direction
(row V = pad row). The g-gate columns/rows are pre-doubled so EVERY
activation in the cell is a sigmoid:

    g = tanh(zg) = 2*sigmoid(2*zg) - 1          (table/W_hh g-rows hold 2*zg)
    tanh(c) = 2*sigmoid(C) - 1  with C := 2c    (state kept doubled)

so one sigmoid covers i,f,o,g-tilde and a second covers the cell tanh:
2 Act ops per chain-step instead of 3, with the corrections folded into
fused scalar_tensor_tensor ops on DVE/GpSimd.

Per chain-step (TensorTensor is not legal on Pool/GpSimd in the V3 ISA
lowering, so all elementwise runs on DVE):
  PE:   8 transpose-injections (token-major gather tile -> gate-major
        PSUM, opening each 32-col accumulation group) + 16 W_hh matmuls.
  Act:  sg = sigmoid(zall [128,256]) f32;  th = tanh(c) f32.
  DVE:  p = i*gt;  m1 = 2p - i (= i*g);  cf = f*c;  c = m1 + cf;
        h = o*th -> bf16 into the h history buffer (doubles as the
        next-step matmul rhs).
The running max is NOT maintained per step: h lands in a [128, 64*64]
history tile per chain and one tensor_reduce(max) per chain at the end
replaces 128 inline max ops.

Gather: 4 batched indirect DMAs (8 tiles x 128 rows each, all issued up
front on the otherwise idle GpSimd queue) into a fully resident
token-major staging buffer st [128, 32*1024] bf16; no PE staging
transposes, no zxT copies.

Scan order: token s of sentence b reads token_ids[b, s] (fwd) or
token_ids[b, len_b-1-s] (bwd), pad row for s >= len_b, so pads always
trail and both directions share the same program.
"""

import numpy as np
import ml_dtypes

import concourse.bass as bass
import concourse.bacc as bacc
import concourse.mybir as mybir
import concourse.tile as tile
from concourse import bass_utils

V, E, HID, B, T = 50000, 300, 256, 256, 64
NCORES = 8
NSC = 64                    # sentences per core (one direction)
NTOK = NSC * T              # 4096 tokens/core
NTT = NTOK // 128           # 32 gather tiles
G4 = 4 * HID                # 1024 gate logits
VP = V + 1                  # table rows (last = pad row)
BIG = 30.0
GB = 1                      # gather tiles per indirect op
NG = NTT // GB              # indirect op count

F32 = mybir.dt.float32
BF16 = mybir.dt.bfloat16
I32 = mybir.dt.int32
AF = mybir.ActivationFunctionType
OP = mybir.AluOpType

bf = ml_dtypes.bfloat16

_CACHE = {}
LAST_RESULTS = None


def _build_program():
    nc = bacc.Bacc(None, target_bir_lowering=False)

    tab_d = nc.dram_tensor("tab", [VP, G4], BF16, kind="ExternalInput")
    idx_d = nc.dram_tensor("idx", [128, NTT], I32, kind="ExternalInput")
    wstat_d = nc.dram_tensor("wstat", [128, 2048], BF16, kind="ExternalInput")
    out_d = nc.dram_tensor("out", [NSC, HID], F32, kind="ExternalOutput")

    with tile.TileContext(nc) as tc:
        with (
            tc.tile_pool(name="const", bufs=1) as cpool,
            tc.tile_pool(name="work", bufs=2) as wpool,
            tc.tile_pool(name="psum", bufs=1, space="PSUM") as ppool,
            tc.tile_pool(name="psumt", bufs=1, space="PSUM") as tpool,
        ):
            wstat_sb = cpool.tile([128, 2048], BF16, tag="wstat")
            idx_sb = cpool.tile([128, NTT], I32, tag="idx")
            st = cpool.tile([128, NTT * G4], BF16, tag="st")
            hist = [cpool.tile([128, T * 64], BF16, tag=f"hist{x}",
                               name=f"hist{x}") for x in range(2)]
            h0 = cpool.tile([128, 64], BF16, tag="h0")
            c_st = [cpool.tile([128, 64], F32, tag=f"c{x}", name=f"c{x}")
                    for x in range(2)]
            rmax = cpool.tile([128, 128], BF16, tag="rmax")
            ident = cpool.tile([128, 128], F32, tag="ident")
            ident_bf = cpool.tile([128, 128], BF16, tag="ident_bf")
            outT = cpool.tile([128, 128], F32, tag="outT")

            nc.sync.dma_start(out=idx_sb[:], in_=idx_d[:, :])
            nc.sync.dma_start(out=wstat_sb[:], in_=wstat_d[:, :])

            from concourse.masks import make_identity
            make_identity(nc, ident[:])
            nc.vector.tensor_copy(out=ident_bf[:], in_=ident[:])

            for x in range(2):
                nc.vector.memset(c_st[x][:], 0.0)
            nc.vector.memset(h0[:], 0.0)

            _st = st[:]

            def emit_gather(gi):
                # one indirect op gathers 128 rows of 1024 bf16 each into
                # the gi-th 1024-col band of st (plain 2D dest AP — a
                # nonstandard AP here breaks DMA descriptor/sem accounting)
                nc.gpsimd.indirect_dma_start(
                    out=st[:, gi * G4:(gi + 1) * G4],
                    out_offset=None,
                    in_=tab_d[:, :],
                    in_offset=bass.IndirectOffsetOnAxis(
                        ap=idx_sb[:, gi:gi + 1], axis=0),
                )

            for gi in range(NG):
                emit_gather(gi)

            sgs = {}

            def emit_front(x, s):
                # PE block + sigma over all gates; needs h(x, s-1)
                par = s % 2
                u = s // 2
                p0 = (s % 2) * 64 + x * 32   # st partition offset
                z = ppool.tile([128, 256], F32, tag=f"z{x}{par}",
                               name=f"z{x}{par}", bufs=1)
                # transpose-injection opens each 32-col accumulation group:
                # full-height matmul st[0:128]^T @ I[:, p0:p0+32] selects
                # token rows p0..p0+32 (== transpose), f32 PSUM out, with
                # the same (128,128) tile config as the W_hh matmuls.
                def h_prev(k):
                    if s == 0:
                        return h0[:, k * 32:(k + 1) * 32]
                    o = (s - 1) * 64 + k * 32
                    return hist[x][:, o:o + 32]

                for ch in range(8):
                    nc.tensor.matmul(
                        z[:, ch * 32:(ch + 1) * 32],
                        lhsT=st[0:128,
                                u * G4 + ch * 128:u * G4 + (ch + 1) * 128],
                        rhs=ident_bf[0:128, p0:p0 + 32],
                        start=True, stop=False, skip_group_check=True)
                    for k in range(2):
                        nc.tensor.matmul(
                            z[:, ch * 32:(ch + 1) * 32],
                            lhsT=wstat_sb[:, (ch * 2 + k) * 128:
                                          (ch * 2 + k + 1) * 128],
                            rhs=h_prev(k),
                            start=False, stop=(k == 1),
                            skip_group_check=True)
                # sg = [i | f | o | gt], each [128, 64] slice, f32
                sg = wpool.tile([128, 256], F32, tag=f"sg{x}",
                                name=f"sg{x}")
                nc.scalar.activation(sg[:], z[:], AF.Sigmoid)
                sgs[x] = sg

            def emit_back(x, s):
                # elementwise cell update + tanh + h write; follows sigma
                sg = sgs[x]
                p = wpool.tile([128, 64], F32, tag=f"p{x}")
                nc.vector.tensor_mul(p[:], sg[:, 0:64], sg[:, 192:256])
                m1 = wpool.tile([128, 64], F32, tag=f"m1{x}")
                nc.vector.scalar_tensor_tensor(
                    m1[:], p[:], 2.0, sg[:, 0:64], OP.mult, OP.subtract)
                cf = wpool.tile([128, 64], F32, tag=f"cf{x}")
                nc.vector.tensor_mul(cf[:], sg[:, 64:128], c_st[x][:])
                nc.vector.tensor_add(c_st[x][:], m1[:], cf[:])
                th = wpool.tile([128, 64], F32, tag=f"th{x}")
                nc.scalar.activation(th[:], c_st[x][:], AF.Tanh)
                nc.vector.tensor_mul(
                    hist[x][:, s * 64:(s + 1) * 64], sg[:, 128:192], th[:])

            # chain B lags half a step so each engine-queue entry has its
            # inputs ready when the in-order engine reaches it
            emit_front(0, 0)
            emit_front(1, 0)
            emit_back(0, 0)
            for s in range(1, T):
                emit_front(0, s)
                emit_back(1, s - 1)
                emit_front(1, s)
                emit_back(0, s)
            emit_back(1, T - 1)

            # ---- output: max over time, transpose, write out ----
            for x in range(2):
                _h = hist[x][:]
                hist3 = bass.AP(tensor=_h.tensor, offset=_h.offset,
                                ap=[_h.ap[0], [1, 64], [64, T]])
                nc.vector.tensor_reduce(
                    rmax[:, x * 64:(x + 1) * 64], hist3,
                    mybir.AxisListType.X, OP.max)
            tp = tpool.tile([128, 128], BF16, tag="tp")
            nc.tensor.transpose(tp[:], rmax[:], ident_bf[:])
            nc.vector.tensor_copy(out=outT[:], in_=tp[:])
            # outT[j = x*64 + k*32 + b, p] -> out[x*32 + b, k*128 + p]
            for x in range(2):
                for k in range(2):
                    out_ap = bass.AP(
                        tensor=out_d[:, :].tensor,
                        offset=(x * 32) * HID + k * 128,
                        ap=[[HID, 32], [1, 128]])
                    nc.sync.dma_start(
                        out=out_ap,
                        in_=outT[x * 64 + k * 32:x * 64 + k * 32 + 32, :])

    nc.finalize()
    return nc


def _host_prep(token_ids, lengths, emb, w_ih_f, w_hh_f, b_f, w_ih_b, w_hh_b,
               b_b):
    # gate chunk order [i0 i1 f0 f1 o0 o1 g0 g1] as torch rows
    ch_rows = [0, 128, 256, 384, 768, 896, 512, 640]
    col_perm = np.concatenate([np.arange(r, r + 128) for r in ch_rows])

    tabs, wstats = {}, {}
    for d in range(2):
        w_ih = w_ih_f if d == 0 else w_ih_b
        whh = w_hh_f if d == 0 else w_hh_b
        bias = b_f if d == 0 else b_b

        zx = emb.astype(np.float32) @ w_ih.T.astype(np.float32) + bias
        zxp = zx[:, col_perm].copy()
        zxp[:, 768:1024] *= 2.0                     # g chunks doubled
        tab = np.empty((VP, G4), dtype=bf)
        tab[:V] = zxp.astype(bf)
        padv = np.empty(G4, dtype=np.float32)       # in permuted chunk order
        padv[0:256] = BIG                           # i -> 1
        padv[256:512] = -BIG                        # f -> 0
        padv[512:768] = BIG                         # o -> 1
        padv[768:1024] = -BIG                       # gt -> 0 (g -> -1)
        tab[V] = padv.astype(bf)
        tabs[d] = tab

        whh2 = whh.astype(np.float32)
        wstat = np.zeros((128, 2048), dtype=bf)
        for ch in range(8):
            scl = 2.0 if ch >= 6 else 1.0
            for k in range(2):
                blk = whh2[ch_rows[ch]:ch_rows[ch] + 128,
                           k * 128:(k + 1) * 128].T * scl
                col = (ch * 2 + k) * 128
                wstat[:, col:col + 128] = blk.astype(bf)
        wstats[d] = wstat

    in_maps = []
    for c in range(NCORES):
        d = 0 if c < 4 else 1
        blk = c % 4
        tok = token_ids[blk * NSC:(blk + 1) * NSC]      # [64, 64]
        ln = lengths[blk * NSC:(blk + 1) * NSC]         # [64]

        ss = np.arange(T)[None, :]                      # [1, T]
        if d == 0:
            pos = ss                                    # fwd: s
        else:
            pos = ln[:, None] - 1 - ss                  # bwd: len-1-s
        valid = ss < ln[:, None]                        # [64, T]
        rows = np.where(valid, np.take_along_axis(
            tok, np.clip(pos, 0, T - 1), axis=1), V)    # [64, T] table rows
        flat = rows.T.reshape(-1)                       # j = s*64 + b
        idx = flat.reshape(NTT, 128).T.astype(np.int32).copy()

        in_maps.append({
            "tab": tabs[d],
            "idx": idx,
            "wstat": wstats[d],
        })
    return in_maps


def kernel(token_ids, lengths, emb, w_ih_f, w_hh_f, b_f, w_ih_b, w_hh_b, b_b):
    global LAST_RESULTS
    if "nc" not in _CACHE:
        _CACHE["nc"] = _build_program()
    nc = _CACHE["nc"]
    in_maps = _host_prep(token_ids, lengths, emb, w_ih_f, w_hh_f, b_f,
                         w_ih_b, w_hh_b, b_b)
    res = bass_utils.run_bass_kernel_spmd(nc, in_maps, list(range(NCORES)))
    LAST_RESULTS = res
    out = np.zeros((B, 2 * HID), np.float32)
    for c in range(NCORES):
        d = 0 if c < 4 else 1
        blk = c % 4
        out[blk * NSC:(blk + 1) * NSC,
            d * HID:(d + 1) * HID] = res.results[c]["out"]
    return out


# revision 20
# speedup vs baseline: 1.1951x; 1.1951x over previous
"""Trainium2 Bass kernel for STSBaselineNet (embed -> biLSTM -> max-pool).

Sharding v9: one LSTM direction per core (cores 0-3 fwd, 4-7 bwd, 64
sentences each, two independent 32-sentence chains pipelined half a step
apart). The input projection emb @ W_ih^T + b is folded on the host into
a [V+1, 1024] bf16 gather table per direction (row V = pad row). The
g-gate columns/rows are pre-doubled so the gate nonlinearities are one
sigmoid: g = tanh(zg) = 2*sigmoid(2*zg) - 1.

The wall time is 64 x (single-chain loop latency); everything is
arranged to minimize that serial loop:

  h(s-1) -> [12 W_hh matmuls for i,f,g] -> sigma(z_ifg 192c)
         -> p = i*gt; m1 = 2p - i; c = m1 + cf   (DVE; cf = f*c ready)
         -> th = tanh(c) -> h = o*th -> next step

Off the critical path: the 8 zx transpose-injections for step s+1 run
during step s's elementwise phase (PSUM "start" lazily zeroes the whole
2KB zero-region, so only the FIRST write per bank-sized PSUM tile sets
start=True and injects can all be hoisted); the o-gate matmuls and
sigma(z_o) run after sigma(z_ifg); the time-max is computed as 4
interleaved partial tensor_reduce ops per chain instead of per-step
running max ops.

Gather: 32 single-tile indirect DMAs up front on the otherwise idle
GpSimd queue into a resident token-major staging buffer st
[128, 32*1024] bf16.

Scan order: token s of sentence b reads token_ids[b, s] (fwd) or
token_ids[b, len_b-1-s] (bwd), pad row for s >= len_b, so pads always
trail and both directions share the same program.
"""

import numpy as np
import ml_dtypes

import concourse.bass as bass
import concourse.bacc as bacc
import concourse.mybir as mybir
import concourse.tile as tile
from concourse import bass_utils

V, E, HID, B, T = 50000, 300, 256, 256, 64
NCORES = 8
NSC = 64                    # sentences per core (one direction)
NTOK = NSC * T              # 4096 tokens/core
NTT = NTOK // 128           # 32 gather tiles
G4 = 4 * HID                # 1024 gate logits
VP = V + 1                  # table rows (last = pad row)
BIG = 30.0

F32 = mybir.dt.float32
BF16 = mybir.dt.bfloat16
I32 = mybir.dt.int32
AF = mybir.ActivationFunctionType
OP = mybir.AluOpType

bf = ml_dtypes.bfloat16

_CACHE = {}
LAST_RESULTS = None


def _build_program():
    nc = bacc.Bacc(None, target_bir_lowering=False)

    tab_d = nc.dram_tensor("tab", [VP, G4], BF16, kind="ExternalInput")
    idx_d = nc.dram_tensor("idx", [128, NTT], I32, kind="ExternalInput")
    wstat_d = nc.dram_tensor("wstat", [128, 2048], BF16, kind="ExternalInput")
    out_d = nc.dram_tensor("out", [NSC, HID], F32, kind="ExternalOutput")

    with tile.TileContext(nc) as tc:
        with (
            tc.tile_pool(name="const", bufs=1) as cpool,
            tc.tile_pool(name="work", bufs=2) as wpool,
            tc.tile_pool(name="psum", bufs=1, space="PSUM") as ppool,
            tc.tile_pool(name="psumt", bufs=1, space="PSUM") as tpool,
        ):
            wstat_sb = cpool.tile([128, 2048], BF16, tag="wstat")
            idx_sb = cpool.tile([128, NTT], I32, tag="idx")
            st = cpool.tile([128, NTT * G4], BF16, tag="st")
            hist = [cpool.tile([128, T * 64], BF16, tag=f"hist{x}",
                               name=f"hist{x}") for x in range(2)]
            h0 = cpool.tile([128, 64], BF16, tag="h0")
            c_st = [cpool.tile([128, 64], F32, tag=f"c{x}", name=f"c{x}")
                    for x in range(2)]
            pm = [cpool.tile([128, 128], BF16, tag=f"pm{k}", name=f"pm{k}")
                  for k in range(4)]
            rmax = cpool.tile([128, 128], BF16, tag="rmax")
            ident = cpool.tile([128, 128], F32, tag="ident")
            ident_bf = cpool.tile([128, 128], BF16, tag="ident_bf")
            outT = cpool.tile([128, 128], F32, tag="outT")

            nc.sync.dma_start(out=idx_sb[:], in_=idx_d[:, :])
            nc.sync.dma_start(out=wstat_sb[:], in_=wstat_d[:, :])

            from concourse.masks import make_identity
            make_identity(nc, ident[:])
            nc.vector.tensor_copy(out=ident_bf[:], in_=ident[:])

            for x in range(2):
                nc.vector.memset(c_st[x][:], 0.0)
            nc.vector.memset(h0[:], 0.0)

            def emit_gather(gi):
                nc.gpsimd.indirect_dma_start(
                    out=st[:, gi * G4:(gi + 1) * G4],
                    out_offset=None,
                    in_=tab_d[:, :],
                    in_offset=bass.IndirectOffsetOnAxis(
                        ap=idx_sb[:, gi:gi + 1], axis=0),
                )

            for gi in range(NTT):
                emit_gather(gi)

            # PSUM z tiles: one full 2KB bank each (cols 0:256 used) so the
            # single start=True write owns the whole zero-region.
            zt = {}

            def ztile(x, par):
                t = ppool.tile([128, 512], F32, tag=f"z{x}{par}",
                               name=f"z{x}{par}", bufs=1)
                zt[(x, par)] = t
                return t

            def emit_inject(x, s):
                # 8 zx transpose-injections for step s (hoisted off the
                # critical path): full-height matmul st^T @ I[:, p0:p0+32]
                # selects token rows p0..p0+32 == transpose, f32 PSUM out.
                par = s % 2
                u = s // 2
                p0 = (s % 2) * 64 + x * 32
                z = ztile(x, par)
                for ch in range(8):
                    nc.tensor.matmul(
                        z[:, ch * 32:(ch + 1) * 32],
                        lhsT=st[0:128,
                                u * G4 + ch * 128:u * G4 + (ch + 1) * 128],
                        rhs=ident_bf[0:128, p0:p0 + 32],
                        start=(ch == 0), stop=False, skip_group_check=True)

            sgs = {}

            def emit_front(x, s):
                # 16 W_hh matmuls (i,f,g chunks first) + split sigmoid
                z = zt[(x, s % 2)]

                def h_prev(k):
                    if s == 0:
                        return h0[:, k * 32:(k + 1) * 32]
                    o = (s - 1) * 64 + k * 32
                    return hist[x][:, o:o + 32]

                for ch in range(8):
                    for k in range(2):
                        nc.tensor.matmul(
                            z[:, ch * 32:(ch + 1) * 32],
                            lhsT=wstat_sb[:, (ch * 2 + k) * 128:
                                          (ch * 2 + k + 1) * 128],
                            rhs=h_prev(k),
                            start=False,
                            stop=(ch == 7 and k == 1),
                            skip_group_check=True)
                # sg = [i | f | gt | o]
                sg = wpool.tile([128, 256], F32, tag=f"sg{x}",
                                name=f"sg{x}")
                nc.scalar.activation(sg[:, 0:192], z[:, 0:192], AF.Sigmoid)
                nc.scalar.activation(sg[:, 192:256], z[:, 192:256],
                                     AF.Sigmoid)
                sgs[x] = sg

            def emit_back(x, s):
                sg = sgs[x]
                cf = wpool.tile([128, 64], F32, tag=f"cf{x}")
                nc.vector.tensor_mul(cf[:], sg[:, 64:128], c_st[x][:])
                p = wpool.tile([128, 64], F32, tag=f"p{x}")
                nc.vector.tensor_mul(p[:], sg[:, 0:64], sg[:, 128:192])
                m1 = wpool.tile([128, 64], F32, tag=f"m1{x}")
                nc.vector.scalar_tensor_tensor(
                    m1[:], p[:], 2.0, sg[:, 0:64], OP.mult, OP.subtract)
                nc.vector.tensor_add(c_st[x][:], m1[:], cf[:])
                th = wpool.tile([128, 64], F32, tag=f"th{x}")
                nc.scalar.activation(th[:], c_st[x][:], AF.Tanh)
                nc.vector.tensor_mul(
                    hist[x][:, s * 64:(s + 1) * 64], sg[:, 192:256], th[:])

            def emit_partial(x, k):
                # max over steps 16k..16k+16 of chain x's h history
                _h = hist[x][:]
                seg = bass.AP(tensor=_h.tensor,
                              offset=_h.offset + k * 16 * 64,
                              ap=[_h.ap[0], [1, 64], [64, 16]])
                nc.vector.tensor_reduce(
                    pm[k][:, x * 64:(x + 1) * 64], seg,
                    mybir.AxisListType.X, OP.max)

            # chain B lags roughly half a loop so each engine-queue entry
            # has its inputs ready when the in-order engine reaches it
            emit_inject(0, 0)
            emit_front(0, 0)
            emit_inject(1, 0)
            emit_inject(0, 1)
            emit_back(0, 0)
            emit_front(1, 0)
            emit_inject(1, 1)
            for s in range(1, T):
                emit_front(0, s)
                if s + 1 < T:
                    emit_inject(0, s + 1)
                emit_back(1, s - 1)
                if s % 16 == 0:
                    emit_partial(1, s // 16 - 1)
                emit_front(1, s)
                if s + 1 < T:
                    emit_inject(1, s + 1)
                emit_back(0, s)
                if s % 16 == 15:
                    emit_partial(0, s // 16)
            emit_back(1, T - 1)
            emit_partial(1, 3)

            # ---- output: fold partials, transpose, write out ----
            nc.vector.tensor_max(pm[0][:], pm[0][:], pm[1][:])
            nc.vector.tensor_max(pm[2][:], pm[2][:], pm[3][:])
            nc.vector.tensor_max(rmax[:], pm[0][:], pm[2][:])

            tp = tpool.tile([128, 128], BF16, tag="tp")
            nc.tensor.transpose(tp[:], rmax[:], ident_bf[:])
            nc.vector.tensor_copy(out=outT[:], in_=tp[:])
            # outT[j = x*64 + k*32 + b, p] -> out[x*32 + b, k*128 + p]
            for x in range(2):
                for k in range(2):
                    out_ap = bass.AP(
                        tensor=out_d[:, :].tensor,
                        offset=(x * 32) * HID + k * 128,
                        ap=[[HID, 32], [1, 128]])
                    nc.sync.dma_start(
                        out=out_ap,
                        in_=outT[x * 64 + k * 32:x * 64 + k * 32 + 32, :])

    nc.finalize()
    return nc


def _host_prep(token_ids, lengths, emb, w_ih_f, w_hh_f, b_f, w_ih_b, w_hh_b,
               b_b):
    # gate chunk order [i0 i1 f0 f1 g0 g1 o0 o1] (natural torch order)
    ch_rows = [0, 128, 256, 384, 512, 640, 768, 896]
    col_perm = np.concatenate([np.arange(r, r + 128) for r in ch_rows])

    tabs, wstats = {}, {}
    for d in range(2):
        w_ih = w_ih_f if d == 0 else w_ih_b
        whh = w_hh_f if d == 0 else w_hh_b
        bias = b_f if d == 0 else b_b

        zx = emb.astype(np.float32) @ w_ih.T.astype(np.float32) + bias
        zxp = zx[:, col_perm].copy()
        zxp[:, 512:768] *= 2.0                      # g chunks doubled
        tab = np.empty((VP, G4), dtype=bf)
        tab[:V] = zxp.astype(bf)
        padv = np.empty(G4, dtype=np.float32)       # in permuted chunk order
        padv[0:256] = BIG                           # i -> 1
        padv[256:512] = -BIG                        # f -> 0
        padv[512:768] = -BIG                        # gt -> 0 (g -> -1)
        padv[768:1024] = BIG                        # o -> 1
        tab[V] = padv.astype(bf)
        tabs[d] = tab

        whh2 = whh.astype(np.float32)
        wstat = np.zeros((128, 2048), dtype=bf)
        for ch in range(8):
            scl = 2.0 if ch in (4, 5) else 1.0
            for k in range(2):
                blk = whh2[ch_rows[ch]:ch_rows[ch] + 128,
                           k * 128:(k + 1) * 128].T * scl
                col = (ch * 2 + k) * 128
                wstat[:, col:col + 128] = blk.astype(bf)
        wstats[d] = wstat

    in_maps = []
    for c in range(NCORES):
        d = 0 if c < 4 else 1
        blk = c % 4
        tok = token_ids[blk * NSC:(blk + 1) * NSC]      # [64, 64]
        ln = lengths[blk * NSC:(blk + 1) * NSC]         # [64]

        ss = np.arange(T)[None, :]                      # [1, T]
        if d == 0:
            pos = ss                                    # fwd: s
        else:
            pos = ln[:, None] - 1 - ss                  # bwd: len-1-s
        valid = ss < ln[:, None]                        # [64, T]
        rows = np.where(valid, np.take_along_axis(
            tok, np.clip(pos, 0, T - 1), axis=1), V)    # [64, T] table rows
        flat = rows.T.reshape(-1)                       # j = s*64 + b
        idx = flat.reshape(NTT, 128).T.astype(np.int32).copy()

        in_maps.append({
            "tab": tabs[d],
            "idx": idx,
            "wstat": wstats[d],
        })
    return in_maps


def kernel(token_ids, lengths, emb, w_ih_f, w_hh_f, b_f, w_ih_b, w_hh_b, b_b):
    global LAST_RESULTS
    if "nc" not in _CACHE:
        _CACHE["nc"] = _build_program()
    nc = _CACHE["nc"]
    in_maps = _host_prep(token_ids, lengths, emb, w_ih_f, w_hh_f, b_f,
                         w_ih_b, w_hh_b, b_b)
    res = bass_utils.run_bass_kernel_spmd(nc, in_maps, list(range(NCORES)))
    LAST_RESULTS = res
    out = np.zeros((B, 2 * HID), np.float32)
    for c in range(NCORES):
        d = 0 if c < 4 else 1
        blk = c % 4
        out[blk * NSC:(blk + 1) * NSC,
            d * HID:(d + 1) * HID] = res.results[c]["out"]
    return out
